# revision 1
# baseline (speedup 1.0000x reference)
"""MoE layer (top-2 of 8 experts, SwiGLU) on 8 Trainium2 NeuronCores.

Strategy (expert-parallel, matching the sharding hint):
  - Host computes the router (gate logits -> top-2 -> softmax) in fp32 numpy,
    exactly mirroring the reference math. This is the "token dispatch" step:
    tokens are gathered per expert on the host (the all-to-all), each core
    gets one expert's weights plus that expert's tokens.
  - Each core runs a dense SwiGLU MLP over its gathered token batch:
        h = silu(x @ w_gate.T) * (x @ w1.T);  y = h @ w2.T
    Matmuls run in bf16 with fp32 PSUM accumulation.
  - Host multiplies each expert's outputs by the combine weights and
    scatter-adds back into token order (the "combine" step).

Layouts are pre-swizzled on host so every DMA is a simple strided copy:
  xt  [128, 8, C]    : x gathered+transposed, d = ko*128 + p
  w1t [128, 8, 4096] : w1.T   (d on partitions)
  wgt [128, 8, 4096] : w_gate.T
  w2t [128, 32, 1024]: w2.T   (d_ff on partitions)
  y   [C, 1024]      : fp32 output (token-major)
"""

import numpy as np
import ml_dtypes

import concourse.bass as bass
import concourse.mybir as mybir
import concourse.tile as tile
from concourse.bass_utils import run_bass_kernel_spmd

# ---------------------------------------------------------------------------
# Workaround for this walrus build: TPB instructions have a single hardware
# wait slot and this walrus refuses any instruction carrying more than one
# sem wait ("Too many sync wait commands"). Post-pass: for every instruction
# with k>1 waits, hoist k-1 waits onto single-wait NOPs on the same engine
# immediately before it. Program-order semantics are identical (the engine
# blocks on each wait in turn before issuing the instruction).
# ---------------------------------------------------------------------------

_ws_counter = [0]


def _split_multi_waits(nc: bass.Bass) -> int:
    n_split = 0
    for f in nc.m.functions:
        for bb in f.blocks:
            new_insts = []
            for inst in bb.instructions:
                si = inst.sync_info
                if si is not None and si.on_wait and len(si.on_wait) > 1:
                    waits = list(si.on_wait)
                    for w in waits[:-1]:
                        _ws_counter[0] += 1
                        n_split += 1
                        new_insts.append(
                            mybir.InstNoOp(
                                name=f"waitsplit-{_ws_counter[0]}",
                                opcode="NoOp",
                                engine=inst.engine,
                                sync_info=mybir.SyncInfo(
                                    on_wait=[w], on_update=[]
                                ),
                                bass_nofuse=True,
                                text_hint="waitsplit",
                            )
                        )
                    si.on_wait = [waits[-1]]
                new_insts.append(inst)
            bb.instructions[:] = new_insts
    return n_split

# ---------------------------------------------------------------------------

D = 1024
DFF = 4096
N_EXPERTS = 8
TOP_K = 2
N_CORES = 8
TB = 512          # token block processed per outer iteration
WG_BUFS = 4
XT_BUFS = 3
W2_BUFS = 12
H_BUFS = 44
PSG_BUFS = 2
PS1_BUFS = 2
PSY_BUFS = 4
KD = D // 128     # 8 contraction tiles over d
NF = DFF // 128   # 32 tiles over d_ff

BF16 = mybir.dt.bfloat16
F32 = mybir.dt.float32
NP_BF16 = ml_dtypes.bfloat16

_NC_CACHE: dict[int, bass.Bass] = {}


def _build_kernel(C: int, repeat: int = 1) -> bass.Bass:
    """Dense SwiGLU MLP over C tokens (C a multiple of 128; blocks of 512
    plus one smaller tail block).

    repeat>1 wraps the whole computation in a hardware For_i loop that
    recomputes the same result `repeat` times — used only for wall-clock
    calibration of per-iteration hardware time."""
    assert C % 128 == 0
    blocks = [TB] * (C // TB)
    r = C % TB
    if r == 128:
        # a 128-wide block is LDW-bound; prefer 384+256 over 512+128
        if blocks:
            blocks = blocks[:-1] + [384, 256]
        else:
            blocks = [128]
    elif r:
        blocks.append(r)
    # Small blocks first: their stage-2 is DMA-paced (little compute per w2
    # chunk), so schedule them where later blocks' stage-1 matmuls can fill
    # the PE gaps.
    blocks.sort()

    nc = bass.Bass()
    xt = nc.dram_tensor("xt", [128, KD, C], BF16, kind="ExternalInput")
    w1t = nc.dram_tensor("w1t", [128, KD, DFF], BF16, kind="ExternalInput")
    wgt = nc.dram_tensor("wgt", [128, KD, DFF], BF16, kind="ExternalInput")
    w2t = nc.dram_tensor("w2t", [128, NF, D], BF16, kind="ExternalInput")
    y = nc.dram_tensor("y", [C, D], F32, kind="ExternalOutput")

    silu = mybir.ActivationFunctionType.Silu

    with tile.TileContext(nc) as tc:
        with (
            tc.tile_pool(name="wres", bufs=1) as wres,
            tc.tile_pool(name="wg", bufs=WG_BUFS) as wgpool,
            tc.tile_pool(name="xt", bufs=XT_BUFS) as xtpool,
            tc.tile_pool(name="hg", bufs=3) as hgpool,
            tc.tile_pool(name="h", bufs=H_BUFS) as hpool,
            tc.tile_pool(name="w2", bufs=W2_BUFS) as w2pool,
            tc.tile_pool(name="yo", bufs=4) as ypool,
            tc.tile_pool(name="ps1", bufs=1, space="PSUM") as psum1,
            tc.tile_pool(name="ps2", bufs=PSY_BUFS, space="PSUM") as psum2,
        ):
            # Resident w1, split into 8 dff-chunks so the first matmuls only
            # wait on the chunk they need (loaded just-in-time in block 0).
            w1_parts = [
                wres.tile([128, KD, 512], BF16, tag=f"w1p{i}", name=f"w1p{i}")
                for i in range(NF // 4)
            ]

            if repeat > 1:
                # calibration mode: load resident w1 once, outside the loop
                for i in range(NF // 4):
                    nc.sync.dma_start(
                        w1_parts[i][:], w1t[:, :, i * 512:(i + 1) * 512]
                    )

            def _trace_body():
              tok0 = 0
              for b, tb in enumerate(blocks):
                xt_sb = xtpool.tile([128, KD, tb], BF16, tag="xt")
                nc.sync.dma_start(xt_sb[:], xt[:, :, tok0:tok0 + tb])

                h_tiles = []
                for dfc in range(NF // 4):
                    if b == 0 and dfc == 0:
                        # Split the first chunk into 4 independent tiles so
                        # the first matmul waits on 256 KB, not 1 MB.
                        wg_pieces = [
                            wgpool.tile([128, KD, 128], BF16, bufs=1,
                                        tag=f"wg0p{i}", name=f"wg0p{i}")
                            for i in range(4)
                        ]
                        for i in range(4):
                            nc.sync.dma_start(
                                wg_pieces[i][:],
                                wgt[:, :, i * 128:(i + 1) * 128],
                            )
                        wg_ch = None
                    else:
                        wg_pieces = None
                        wg_ch = wgpool.tile([128, KD, 512], BF16, tag="wg")
                        nc.sync.dma_start(
                            wg_ch[:], wgt[:, :, dfc * 512:(dfc + 1) * 512]
                        )
                    if b == 0 and repeat == 1:
                        nc.sync.dma_start(
                            w1_parts[dfc][:],
                            w1t[:, :, dfc * 512:(dfc + 1) * 512],
                        )
                    for j in range(4):
                        df = dfc * 4 + j
                        psg = psum1.tile([128, tb], F32, tag="psg", bufs=PSG_BUFS)
                        for d in range(KD):
                            if wg_pieces is not None:
                                wslice = wg_pieces[j][:, d, :]
                            else:
                                wslice = wg_ch[:, d, j * 128:(j + 1) * 128]
                            nc.tensor.matmul(
                                psg[:],
                                wslice,
                                xt_sb[:, d, :],
                                start=(d == 0),
                                stop=(d == KD - 1),
                            )
                        ps1t = psum1.tile([128, tb], F32, tag="ps1t", bufs=PS1_BUFS)
                        for d in range(KD):
                            nc.tensor.matmul(
                                ps1t[:],
                                w1_parts[dfc][:, d, j * 128:(j + 1) * 128],
                                xt_sb[:, d, :],
                                start=(d == 0),
                                stop=(d == KD - 1),
                            )
                        hg = hgpool.tile([128, tb], BF16, tag="hg")
                        nc.scalar.activation(hg[:], psg[:], silu)
                        h = hpool.tile([128, tb], BF16, tag="h")
                        nc.vector.tensor_mul(h[:], hg[:], ps1t[:])
                        h_tiles.append(h)

                n_m = tb // 128
                for half in range(2):
                    psys = [
                        psum2.tile([128, 512], F32, tag="psy", name=f"psy{m}")
                        for m in range(n_m)
                    ]
                    for df in range(NF):
                        w2_ch = w2pool.tile([128, 512], BF16, tag="w2c")
                        nc.sync.dma_start(
                            w2_ch[:], w2t[:, df, half * 512:(half + 1) * 512]
                        )
                        for m in range(n_m):
                            nc.tensor.matmul(
                                psys[m][:],
                                h_tiles[df][:, m * 128:(m + 1) * 128],
                                w2_ch[:],
                                start=(df == 0),
                                stop=(df == NF - 1),
                            )
                    for m in range(n_m):
                        y_sb = ypool.tile([128, 512], F32, tag="ysb")
                        nc.vector.tensor_copy(y_sb[:], psys[m][:])
                        nc.sync.dma_start(
                            y[
                                tok0 + m * 128: tok0 + (m + 1) * 128,
                                half * 512:(half + 1) * 512,
                            ],
                            y_sb[:],
                        )
                tok0 += tb

            if repeat == 1:
                _trace_body()
            else:
                with tc.For_i(0, repeat, 1):
                    _trace_body()
    _split_multi_waits(nc)
    return nc


def _swizzle_k(a: np.ndarray) -> np.ndarray:
    """[K, F] -> [128, K//128, F] with K = ko*128 + p on partitions."""
    k, f = a.shape
    return np.ascontiguousarray(
        a.reshape(k // 128, 128, f).transpose(1, 0, 2)
    )


def kernel(x, gate_w, w1, w_gate, w2):
    b, t, d = x.shape
    xf = np.ascontiguousarray(x.reshape(-1, d)).astype(np.float32)
    n_tok = xf.shape[0]

    # --- Router (host, fp32, mirrors reference math) ---
    logits = xf @ gate_w.T.astype(np.float32)                  # [N, E]
    top_idx = np.argsort(-logits, axis=1, kind="stable")[:, :TOP_K]  # [N, K]
    top_vals = np.take_along_axis(logits, top_idx, axis=1)
    m = top_vals.max(axis=1, keepdims=True)
    ex = np.exp(top_vals - m)
    top_w = ex / ex.sum(axis=1, keepdims=True)                 # [N, K]

    pair_expert = top_idx.reshape(-1)                          # [N*K]
    pair_w = top_w.reshape(-1).astype(np.float32)
    order = np.argsort(pair_expert, kind="stable")
    counts = np.bincount(pair_expert, minlength=N_EXPERTS)
    starts = np.concatenate([[0], np.cumsum(counts)])

    C = max(128, int(-(-int(counts.max()) // 128)) * 128)

    # --- Build per-core inputs (dispatch) ---
    in_maps = []
    sels = []
    for e in range(N_EXPERTS):
        sel = order[starts[e]:starts[e + 1]]
        sels.append(sel)
        tok = sel // TOP_K
        xt_full = np.zeros((D, C), dtype=np.float32)
        xt_full[:, : len(tok)] = xf[tok].T
        in_maps.append(
            {
                "xt": _swizzle_k(xt_full).astype(NP_BF16),
                "w1t": _swizzle_k(
                    np.ascontiguousarray(w1[e].T).astype(np.float32)
                ).astype(NP_BF16),
                "wgt": _swizzle_k(
                    np.ascontiguousarray(w_gate[e].T).astype(np.float32)
                ).astype(NP_BF16),
                "w2t": _swizzle_k(
                    np.ascontiguousarray(w2[e].T).astype(np.float32)
                ).astype(NP_BF16),
            }
        )

    if C not in _NC_CACHE:
        _NC_CACHE[C] = _build_kernel(C)
    nc = _NC_CACHE[C]

    res = run_bass_kernel_spmd(nc, in_maps, core_ids=list(range(N_CORES)))

    # --- Combine (host): weight by router prob, scatter-add to tokens ---
    contrib = np.zeros((n_tok * TOP_K, D), dtype=np.float32)
    for e in range(N_EXPERTS):
        sel = sels[e]
        y_e = res.results[e]["y"][: len(sel)]
        contrib[sel] = y_e * pair_w[sel][:, None]
    out = contrib.reshape(n_tok, TOP_K, D).sum(axis=1)
    return out.reshape(b, t, d).astype(x.dtype)



# revision 5
# speedup vs baseline: 1.5663x; 1.5663x over previous
"""MoE layer (top-2 of 8 experts, SwiGLU) on 8 Trainium2 NeuronCores.

Strategy (expert-parallel, matching the sharding hint):
  - Host computes the router (gate logits -> top-2 -> softmax) in fp32 numpy,
    exactly mirroring the reference math. This is the "token dispatch" step:
    tokens are gathered per expert on the host (the all-to-all), each core
    gets one expert's weights plus that expert's tokens.
  - Each core runs a dense SwiGLU MLP over its gathered token batch:
        h = silu(x @ w_gate.T) * (x @ w1.T);  y = h @ w2.T
    Matmuls run in bf16 with fp32 PSUM accumulation.
  - Host multiplies each expert's outputs by the combine weights and
    scatter-adds back into token order (the "combine" step).

Layouts are pre-swizzled on host so every DMA is a simple strided copy:
  xt  [128, 8, C]    : x gathered+transposed, d = ko*128 + p
  w1t [128, 8, 4096] : w1.T   (d on partitions)
  wgt [128, 8, 4096] : w_gate.T
  w2t [128, 32, 1024]: w2.T   (d_ff on partitions)
  y   [C, 1024]      : fp32 output (token-major)
"""

import numpy as np
import ml_dtypes

import concourse.bass as bass
import concourse.mybir as mybir
import concourse.tile as tile
from concourse.bass_utils import run_bass_kernel_spmd

# ---------------------------------------------------------------------------
# Workaround for this walrus build: TPB instructions have a single hardware
# wait slot and this walrus refuses any instruction carrying more than one
# sem wait ("Too many sync wait commands"). Post-pass: for every instruction
# with k>1 waits, hoist k-1 waits onto single-wait NOPs on the same engine
# immediately before it. Program-order semantics are identical (the engine
# blocks on each wait in turn before issuing the instruction).
# ---------------------------------------------------------------------------

_ws_counter = [0]


def _split_multi_waits(nc: bass.Bass) -> int:
    n_split = 0
    for f in nc.m.functions:
        for bb in f.blocks:
            new_insts = []
            for inst in bb.instructions:
                si = inst.sync_info
                if si is not None and si.on_wait and len(si.on_wait) > 1:
                    waits = list(si.on_wait)
                    for w in waits[:-1]:
                        _ws_counter[0] += 1
                        n_split += 1
                        new_insts.append(
                            mybir.InstNoOp(
                                name=f"waitsplit-{_ws_counter[0]}",
                                opcode="NoOp",
                                engine=inst.engine,
                                sync_info=mybir.SyncInfo(
                                    on_wait=[w], on_update=[]
                                ),
                                bass_nofuse=True,
                                text_hint="waitsplit",
                            )
                        )
                    si.on_wait = [waits[-1]]
                new_insts.append(inst)
            bb.instructions[:] = new_insts
    return n_split

# ---------------------------------------------------------------------------

D = 1024
DFF = 4096
N_EXPERTS = 8
TOP_K = 2
N_CORES = 8
TB = 512          # token block processed per outer iteration
WG_BUFS = 4
XT_BUFS = 3
W2_BUFS = 12
H_BUFS = 44
PSG_BUFS = 2
PS1_BUFS = 2
PSY_BUFS = 4
KD = D // 128     # 8 contraction tiles over d
NF = DFF // 128   # 32 tiles over d_ff

BF16 = mybir.dt.bfloat16
F32 = mybir.dt.float32
NP_BF16 = ml_dtypes.bfloat16

_NC_CACHE: dict[int, bass.Bass] = {}


def _build_kernel(C: int, repeat: int = 1, variant: int = 0) -> bass.Bass:
    """Dense SwiGLU MLP over C tokens (C a multiple of 128; blocks of 512
    plus one smaller tail block).

    repeat>1 wraps the whole computation in a hardware For_i loop that
    recomputes the same result `repeat` times — used only for wall-clock
    calibration of per-iteration hardware time."""
    assert C % 128 == 0
    blocks = [TB] * (C // TB)
    r = C % TB
    if r == 128:
        # a 128-wide block is LDW-bound; prefer 384+256 over 512+128
        if blocks:
            blocks = blocks[:-1] + [384, 256]
        else:
            blocks = [128]
    elif r:
        blocks.append(r)
    # Small blocks first: their stage-2 is DMA-paced (little compute per w2
    # chunk), so schedule them where later blocks' stage-1 matmuls can fill
    # the PE gaps.
    blocks.sort()

    nc = bass.Bass()
    xt = nc.dram_tensor("xt", [128, KD, C], BF16, kind="ExternalInput")
    w1t = nc.dram_tensor("w1t", [128, KD, DFF], BF16, kind="ExternalInput")
    wgt = nc.dram_tensor("wgt", [128, KD, DFF], BF16, kind="ExternalInput")
    w2t = nc.dram_tensor("w2t", [128, NF, D], BF16, kind="ExternalInput")
    y = nc.dram_tensor("y", [C, D], F32, kind="ExternalOutput")

    silu = mybir.ActivationFunctionType.Silu

    with tile.TileContext(nc) as tc:
        with (
            tc.tile_pool(name="wres", bufs=1) as wres,
            tc.tile_pool(name="wg", bufs=WG_BUFS) as wgpool,
            tc.tile_pool(name="xt", bufs=XT_BUFS) as xtpool,
            tc.tile_pool(name="hg", bufs=3) as hgpool,
            tc.tile_pool(name="h", bufs=H_BUFS) as hpool,
            tc.tile_pool(name="w2", bufs=W2_BUFS) as w2pool,
            tc.tile_pool(name="yo", bufs=4) as ypool,
            tc.tile_pool(name="ps1", bufs=1, space="PSUM") as psum1,
            tc.tile_pool(name="ps2", bufs=PSY_BUFS, space="PSUM") as psum2,
        ):
            # Resident w1, split into 8 dff-chunks so the first matmuls only
            # wait on the chunk they need (loaded just-in-time in block 0).
            w1_parts = [
                wres.tile([128, KD, 512], BF16, tag=f"w1p{i}", name=f"w1p{i}")
                for i in range(NF // 4)
            ]

            if repeat > 1:
                # calibration mode: load resident w1 once, outside the loop
                for i in range(NF // 4):
                    nc.sync.dma_start(
                        w1_parts[i][:], w1t[:, :, i * 512:(i + 1) * 512]
                    )

            def _trace_body():
              tok0 = 0
              for b, tb in enumerate(blocks):
                xt_sb = xtpool.tile([128, KD, tb], BF16, tag="xt")
                nc.sync.dma_start(xt_sb[:], xt[:, :, tok0:tok0 + tb])

                h_tiles = []
                for dfc in range(NF // 4):
                    if b == 0 and dfc == 0:
                        # Split the first chunk into 4 independent tiles so
                        # the first matmul waits on 256 KB, not 1 MB.
                        wg_pieces = [
                            wgpool.tile([128, KD, 128], BF16, bufs=1,
                                        tag=f"wg0p{i}", name=f"wg0p{i}")
                            for i in range(4)
                        ]
                        for i in range(4):
                            nc.sync.dma_start(
                                wg_pieces[i][:],
                                wgt[:, :, i * 128:(i + 1) * 128],
                            )
                        wg_ch = None
                    else:
                        wg_pieces = None
                        wg_ch = wgpool.tile([128, KD, 512], BF16, tag="wg")
                        nc.sync.dma_start(
                            wg_ch[:], wgt[:, :, dfc * 512:(dfc + 1) * 512]
                        )
                    if b == 0 and repeat == 1:
                        nc.sync.dma_start(
                            w1_parts[dfc][:],
                            w1t[:, :, dfc * 512:(dfc + 1) * 512],
                        )
                    for j in range(4):
                        df = dfc * 4 + j
                        psg = psum1.tile([128, tb], F32, tag="psg", bufs=PSG_BUFS)
                        if variant == 1:
                            # interleave the two accumulation chains
                            ps1t = psum1.tile([128, tb], F32, tag="ps1t", bufs=PS1_BUFS)
                            for d in range(KD):
                                if wg_pieces is not None:
                                    wslice = wg_pieces[j][:, d, :]
                                else:
                                    wslice = wg_ch[:, d, j * 128:(j + 1) * 128]
                                nc.tensor.matmul(
                                    psg[:], wslice, xt_sb[:, d, :],
                                    start=(d == 0), stop=(d == KD - 1),
                                )
                                nc.tensor.matmul(
                                    ps1t[:],
                                    w1_parts[dfc][:, d, j * 128:(j + 1) * 128],
                                    xt_sb[:, d, :],
                                    start=(d == 0), stop=(d == KD - 1),
                                )
                        else:
                            for d in range(KD):
                                if wg_pieces is not None:
                                    wslice = wg_pieces[j][:, d, :]
                                else:
                                    wslice = wg_ch[:, d, j * 128:(j + 1) * 128]
                                nc.tensor.matmul(
                                    psg[:],
                                    wslice,
                                    xt_sb[:, d, :],
                                    start=(d == 0),
                                    stop=(d == KD - 1),
                                )
                            ps1t = psum1.tile([128, tb], F32, tag="ps1t", bufs=PS1_BUFS)
                            for d in range(KD):
                                nc.tensor.matmul(
                                    ps1t[:],
                                    w1_parts[dfc][:, d, j * 128:(j + 1) * 128],
                                    xt_sb[:, d, :],
                                    start=(d == 0),
                                    stop=(d == KD - 1),
                                )
                        hg = hgpool.tile([128, tb], BF16, tag="hg")
                        nc.scalar.activation(hg[:], psg[:], silu)
                        h = hpool.tile([128, tb], BF16, tag="h")
                        nc.vector.tensor_mul(h[:], hg[:], ps1t[:])
                        h_tiles.append(h)

                n_m = tb // 128
                for half in range(2):
                    psys = [
                        psum2.tile([128, 512], F32, tag="psy", name=f"psy{m}")
                        for m in range(n_m)
                    ]
                    for df in range(NF):
                        w2_ch = w2pool.tile([128, 512], BF16, tag="w2c")
                        nc.sync.dma_start(
                            w2_ch[:], w2t[:, df, half * 512:(half + 1) * 512]
                        )
                        for m in range(n_m):
                            nc.tensor.matmul(
                                psys[m][:],
                                h_tiles[df][:, m * 128:(m + 1) * 128],
                                w2_ch[:],
                                start=(df == 0),
                                stop=(df == NF - 1),
                            )
                    for m in range(n_m):
                        y_sb = ypool.tile([128, 512], F32, tag="ysb")
                        nc.vector.tensor_copy(y_sb[:], psys[m][:])
                        nc.sync.dma_start(
                            y[
                                tok0 + m * 128: tok0 + (m + 1) * 128,
                                half * 512:(half + 1) * 512,
                            ],
                            y_sb[:],
                        )
                tok0 += tb

            if repeat == 1:
                _trace_body()
            else:
                with tc.For_i(0, repeat, 1):
                    _trace_body()
    _split_multi_waits(nc)
    return nc


def _swizzle_k(a: np.ndarray) -> np.ndarray:
    """[K, F] -> [128, K//128, F] with K = ko*128 + p on partitions."""
    k, f = a.shape
    return np.ascontiguousarray(
        a.reshape(k // 128, 128, f).transpose(1, 0, 2)
    )


def kernel(x, gate_w, w1, w_gate, w2):
    b, t, d = x.shape
    xf = np.ascontiguousarray(x.reshape(-1, d)).astype(np.float32)
    n_tok = xf.shape[0]

    # --- Router (host, fp32, mirrors reference math) ---
    logits = xf @ gate_w.T.astype(np.float32)                  # [N, E]
    top_idx = np.argsort(-logits, axis=1, kind="stable")[:, :TOP_K]  # [N, K]
    top_vals = np.take_along_axis(logits, top_idx, axis=1)
    m = top_vals.max(axis=1, keepdims=True)
    ex = np.exp(top_vals - m)
    top_w = ex / ex.sum(axis=1, keepdims=True)                 # [N, K]

    pair_expert = top_idx.reshape(-1)                          # [N*K]
    pair_w = top_w.reshape(-1).astype(np.float32)
    order = np.argsort(pair_expert, kind="stable")
    counts = np.bincount(pair_expert, minlength=N_EXPERTS)
    starts = np.concatenate([[0], np.cumsum(counts)])

    C = max(128, int(-(-int(counts.max()) // 128)) * 128)

    # --- Build per-core inputs (dispatch) ---
    in_maps = []
    sels = []
    for e in range(N_EXPERTS):
        sel = order[starts[e]:starts[e + 1]]
        sels.append(sel)
        tok = sel // TOP_K
        xt_full = np.zeros((D, C), dtype=np.float32)
        xt_full[:, : len(tok)] = xf[tok].T
        in_maps.append(
            {
                "xt": _swizzle_k(xt_full).astype(NP_BF16),
                "w1t": _swizzle_k(
                    np.ascontiguousarray(w1[e].T).astype(np.float32)
                ).astype(NP_BF16),
                "wgt": _swizzle_k(
                    np.ascontiguousarray(w_gate[e].T).astype(np.float32)
                ).astype(NP_BF16),
                "w2t": _swizzle_k(
                    np.ascontiguousarray(w2[e].T).astype(np.float32)
                ).astype(NP_BF16),
            }
        )

    if C not in _NC_CACHE:
        _NC_CACHE[C] = _build_kernel(C)
    nc = _NC_CACHE[C]

    res = run_bass_kernel_spmd(nc, in_maps, core_ids=list(range(N_CORES)))

    # --- Combine (host): weight by router prob, scatter-add to tokens ---
    contrib = np.zeros((n_tok * TOP_K, D), dtype=np.float32)
    for e in range(N_EXPERTS):
        sel = sels[e]
        y_e = res.results[e]["y"][: len(sel)]
        contrib[sel] = y_e * pair_w[sel][:, None]
    out = contrib.reshape(n_tok, TOP_K, D).sum(axis=1)
    return out.reshape(b, t, d).astype(x.dtype)

